# revision 9
# baseline (speedup 1.0000x reference)
"""Trainium2 Bass kernel for nn_Add_PairLinears.

y = sum_a( blockdiag2(W[a]) applied to x[:, perms[a]] ) + sum_a b[a]

Strategy (data-parallel over batch, 8 cores, no collectives):
  - Each core owns a batch shard of 1024 rows, processed as two halves
    of HB=512 batch columns (after transposing to x^T).
  - Phase 1 (per half): load x [128b, 1024d] f32 tiles, PE-transpose the
    f32 data directly (2 cyc/row), convert to bf16 in the DVE PSUM->SBUF
    staging copy, stage full j-groups [128, 8, HB] in SBUF, spill to
    DRAM x^T with 1KB-per-partition-row descriptors. The staged tiles
    stay resident and serve as the identity mixer's (a=0) operands.
  - lhsT (block-diagonal weights expanded to 128x128 tiles) is built ON
    DEVICE from a 128KB compact table: lhsT[t, j, a, 2m+oo] =
    maskc[t, m] * Wc[t, j, a, oo], via per-partition-scalar multiplies
    split across the DVE / gpsimd / scalar engines. Saves 8.4MB of DMA.
  - Gather (per half, per output j-tile): one SWDGE dma_gather covering
    all 7 permuted mixers (num_idxs=896 <= descriptor-scratch limit),
    round-robined over SWDGE queues 1..3 (queue 0 carries the x^T
    spills).
  - Mix: per output j-tile, 8 accumulating 128x128 bf16 matmuls into a
    PSUM bank ([128, 512] f32); evacuated with the per-partition bias
    sum_a b[a] fused (alternating scalar/DVE), stored as y^T bf16 with
    one batched DMA per 4-j group on the scalar HWDGE queue.
"""

import os

import numpy as np
import ml_dtypes

import concourse.bass as bass
import concourse.bacc as bacc
import concourse.tile as tile
from concourse import library_config, mybir
from concourse.bass_utils import run_bass_kernel_spmd

B, D, A = 8192, 4096, 8
N_CORES = 8
BC = B // N_CORES          # 1024 batch rows per core
HB = BC // 2               # 512-wide batch half
NJ = D // 128              # 32 d-tiles of 128
JG = 4                     # j-tiles per mix group
NG = NJ // JG              # mix groups per half (8)
JSP = 8                    # j-tiles per phase-1 staging group
NSP = NJ // JSP            # staging groups per half (4)
NQ = 4                     # SWDGE queues
GCH = int(os.environ.get("GCH", "1"))  # j-tiles per gather call
JW = (A - 1) * 128 // 16   # idx columns per j-tile (56)

F32 = mybir.dt.float32
BF16 = mybir.dt.bfloat16
I16 = mybir.dt.int16

_GRAPH_CACHE = {}
_LAST_RESULTS = None


def _build_graph():
    nc = bacc.Bacc(None, num_swdge_queues=NQ)

    x_ext = nc.declare_dram_parameter("x", [BC, D], F32, isOutput=False)
    wc_ext = nc.declare_dram_parameter("wc", [128, NJ * A * 2], F32, isOutput=False)
    maskc_ext = nc.declare_dram_parameter("maskc", [128, 64], BF16, isOutput=False)
    idx_ext = nc.declare_dram_parameter("idx", [128, NJ * JW], I16, isOutput=False)
    bsum_ext = nc.declare_dram_parameter("bsum", [128, NJ], F32, isOutput=False)
    ident_ext = nc.declare_dram_parameter("ident", [128, 128], F32, isOutput=False)
    yt_ext = nc.declare_dram_parameter("yt", [D, BC], BF16, isOutput=True)

    qn = [0]

    def next_q():
        q = qn[0]
        qn[0] = (q + 1) % 3
        return 1 + q   # gathers use queues 1..3; spills own queue 0

    with tile.TileContext(nc) as tc:
        with (
            tc.tile_pool(name="const", bufs=1) as constp,
            tc.tile_pool(name="xin", bufs=4) as xinp,
            tc.tile_pool(name="st", bufs=6) as stp,
            tc.tile_pool(name="lhs", bufs=1) as lhsp,
            tc.tile_pool(name="g", bufs=2) as gthp,
            tc.tile_pool(name="y", bufs=2) as ybp,
            tc.tile_pool(name="pst", bufs=2, space="PSUM") as pstp,
            tc.tile_pool(name="psm", bufs=4, space="PSUM") as psmp,
            tc.tile_pool(name="dram", bufs=1, space="DRAM") as dramp,
        ):
            nc.gpsimd.load_library(library_config.mlp)

            ident = constp.tile([128, 128], F32)
            nc.sync.dma_start(out=ident[:], in_=ident_ext[:])
            maskc = constp.tile([128, 64], BF16)
            nc.sync.dma_start(out=maskc[:], in_=maskc_ext[:])
            wc_sb = constp.tile([128, NJ, A, 2], F32)
            nc.sync.dma_start(
                out=wc_sb[:],
                in_=wc_ext[:].rearrange("t (j a o) -> t j (a o)", j=NJ, a=A))
            idx_sb = constp.tile([128, NJ * JW], I16)
            nc.scalar.dma_start(out=idx_sb[:], in_=idx_ext[:])
            bsum_sb = constp.tile([128, NJ], F32)
            nc.scalar.dma_start(out=bsum_sb[:], in_=bsum_ext[:])

            # expand the compact weight table into block-diagonal lhsT tiles:
            # lhs_all[t, j, a*128 + 2m+oo] = maskc[t, m] * wc[t, j, a, oo]
            lhs_all = lhsp.tile([128, NJ, A * 128], BF16)
            lhs_v = lhs_all[:].rearrange("t j (a m o) -> t j a m o", a=A, o=2)
            eng = [nc.vector, nc.gpsimd, nc.scalar]
            for j in range(NJ):
                for a in range(A):
                    for oo in range(2):
                        e = eng[(j * A * 2 + a * 2 + oo) % 3]
                        out = lhs_v[:, j, a, :, oo]
                        w = wc_sb[:, j, a, oo:oo + 1]
                        if e is nc.scalar:
                            e.activation(
                                out, maskc[:],
                                mybir.ActivationFunctionType.Copy, scale=w)
                        else:
                            e.tensor_scalar_mul(out, maskc[:], w)

            yt_v = yt_ext[:].rearrange("(j p) b -> p j b", p=128)

            # per-half x^T DRAM buffers (rows of HB for the gathers)
            xt_d0 = dramp.tile([D, HB], BF16, tag="xt0")
            xt_d1 = dramp.tile([D, HB], BF16, tag="xt1")
            xt_d = [xt_d0, xt_d1]
            xt_v = [t[:].rearrange("(q p) b -> p q b", p=128) for t in xt_d]

            # staged x^T tiles, kept for the identity mixer: st[h][sg]
            st_tiles = [[None] * NSP for _ in range(2)]

            def phase1_group(h, sg):
                """load + PE transpose (f32) for j-tiles [8sg, 8sg+8) of
                batch half h; stage in SBUF (bf16) and spill to DRAM x^T."""
                st = stp.tile([128, JSP, HB], BF16, tag="st")
                st_tiles[h][sg] = st
                for bt in range(HB // 128):
                    b0 = h * HB + bt * 128
                    xin = xinp.tile([128, JSP * 128], F32, tag="xin")
                    nc.sync.dma_start(
                        out=xin[:],
                        in_=x_ext[b0:b0 + 128,
                                  sg * JSP * 128:(sg + 1) * JSP * 128])
                    pt = pstp.tile([128, JSP, 128], F32, tag="pst")
                    for jj in range(JSP):
                        nc.tensor.transpose(
                            pt[:, jj, :], xin[:, jj * 128:(jj + 1) * 128],
                            ident[:])
                    nc.vector.tensor_copy(
                        st[:, :, bt * 128:(bt + 1) * 128], pt[:])
                nc.gpsimd.dma_start(
                    out=xt_v[h][:, sg * JSP:(sg + 1) * JSP, :], in_=st[:])

            def mix_group(h, g):
                """gather (mixers 1..7) + mix + store for group g of half h."""
                gt = gthp.tile([128, JG * (A - 1), HB], BF16, tag="g")
                for c0 in range(0, JG, GCH):
                    j0 = g * JG + c0
                    nc.gpsimd.dma_gather(
                        out_ap=gt[:, c0 * (A - 1):(c0 + GCH) * (A - 1), :],
                        in_ap=xt_d[h][:],
                        idxs_ap=idx_sb[:, j0 * JW:(j0 + GCH) * JW],
                        num_idxs=GCH * (A - 1) * 128,
                        num_idxs_reg=GCH * (A - 1) * 128,
                        elem_size=HB,
                        queue_num=next_q(),
                    )
                st = st_tiles[h][g // (JSP // JG)]
                yb = ybp.tile([128, JG, HB], BF16, tag="y")
                for jc in range(JG):
                    j = g * JG + jc
                    pm = psmp.tile([128, HB], F32, tag="psm")
                    for a in range(A):
                        if a == 0:
                            rhs = st[:, (g % (JSP // JG)) * JG + jc, :]
                        else:
                            rhs = gt[:, jc * (A - 1) + (a - 1), :]
                        nc.tensor.matmul(
                            pm[:],
                            lhs_all[:, j, a * 128:(a + 1) * 128],
                            rhs,
                            start=(a == 0),
                            stop=(a == A - 1),
                        )
                    if jc % 2 == 0:
                        nc.scalar.activation(
                            yb[:, jc, :],
                            pm[:],
                            mybir.ActivationFunctionType.Identity,
                            bias=bsum_sb[:, j:j + 1],
                        )
                    else:
                        nc.vector.tensor_scalar_add(
                            yb[:, jc, :], pm[:], bsum_sb[:, j:j + 1])
                nc.scalar.dma_start(
                    out=yt_v[:, g * JG:(g + 1) * JG, h * HB:(h + 1) * HB],
                    in_=yb[:])

            for h in range(2):
                for sg in range(NSP):
                    phase1_group(h, sg)
            for h in range(2):
                for g in range(NG):
                    mix_group(h, g)

    nc.compile()
    return nc


def _host_tables(W, b, perms):
    """Build the device-side constant tables from W/b/perms."""
    # lhsT[j, t, a, o]: weight applied to gathered row t (= x^T[perms[a, 128j+t]])
    # contributing to output row 128j+o.  Output 2n+oo uses inputs
    # perms[a, 2n+i] with weight W[a, n, i, oo]; within tile j, t = 2m+i,
    # o = 2m+oo for pair m = n - 64j.
    Wr = W.reshape(A, NJ, 64, 2, 2)
    lhsT = np.zeros((NJ, 128, A, 128), np.float32)
    m = np.arange(64)
    for i in range(2):
        for oo in range(2):
            # paired advanced indexing on axes 1 and 3 -> result axes [64, NJ, A]
            lhsT[:, 2 * m + i, :, 2 * m + oo] = Wr[:, :, :, i, oo].transpose(2, 1, 0)
    lhsT = np.ascontiguousarray(lhsT.reshape(NJ, 128, A * 128)).astype(ml_dtypes.bfloat16)

    # idx: per output j-tile, the concatenation over mixers a=1..7 of
    # perms[a, 128j : 128(j+1)], wrapped over 16 partitions (index i at
    # [i%16, i//16]) and replicated into each Q7 core's 16-partition group
    idx = np.zeros((128, NJ * JW), np.int16)
    for j in range(NJ):
        vec = np.concatenate([
            perms[a, j * 128:(j + 1) * 128] for a in range(1, A)
        ]).astype(np.int16)
        w16 = vec.reshape(JW, 16).T
        idx[:, j * JW:(j + 1) * JW] = np.tile(w16, (8, 1))

    bsum = np.ascontiguousarray(
        b.astype(np.float64).sum(axis=0).astype(np.float32).reshape(NJ, 128).T)
    ident = np.eye(128, dtype=np.float32)
    return lhsT, idx, bsum, ident


def _compact_tables(W):
    """Compact weight table + pair mask for on-device lhsT expansion."""
    Wr = W.reshape(A, NJ, 64, 2, 2)
    # Wc[t, j, a, oo] = W[a, 64j + t//2, t%2, oo]
    Wc = np.ascontiguousarray(Wr.transpose(2, 3, 1, 0, 4).reshape(128, NJ * A * 2))
    Wc = Wc.astype(ml_dtypes.bfloat16).astype(np.float32)
    t = np.arange(128)
    maskc = (np.arange(64)[None, :] == (t[:, None] // 2)).astype(ml_dtypes.bfloat16)
    return Wc, np.ascontiguousarray(maskc)


def kernel(x, W, b, perms):
    x = np.asarray(x, dtype=np.float32)
    W = np.asarray(W, dtype=np.float32)
    b = np.asarray(b, dtype=np.float32)
    perms = np.asarray(perms)

    _, idx, bsum, ident = _host_tables(W, b, perms)
    Wc, maskc = _compact_tables(W)

    if "nc" not in _GRAPH_CACHE:
        _GRAPH_CACHE["nc"] = _build_graph()
    nc = _GRAPH_CACHE["nc"]

    in_maps = []
    for c in range(N_CORES):
        in_maps.append({
            "x": np.ascontiguousarray(x[c * BC:(c + 1) * BC]),
            "wc": Wc,
            "maskc": maskc,
            "idx": idx,
            "bsum": bsum,
            "ident": ident,
        })

    res = run_bass_kernel_spmd(nc, in_maps, core_ids=list(range(N_CORES)))
    global _LAST_RESULTS
    _LAST_RESULTS = res
    y = np.concatenate(
        [np.asarray(res.results[c]["yt"], dtype=np.float32).T for c in range(N_CORES)],
        axis=0,
    )
    return np.ascontiguousarray(y)


# revision 10
# speedup vs baseline: 1.3757x; 1.3757x over previous
"""Trainium2 Bass kernel for nn_Add_PairLinears.

y = sum_a( blockdiag2(W[a]) applied to x[:, perms[a]] ) + sum_a b[a]

Strategy (data-parallel over batch, 8 cores, no collectives):
  - Each core owns a batch shard of 1024 rows, processed as two halves
    of HB=512 batch columns (after transposing to x^T).
  - Phase 1 (per half): load x [128b, 1024d] f32 tiles, PE-transpose the
    f32 data directly (2 cyc/row), convert to bf16 in the DVE PSUM->SBUF
    staging copy, stage full j-groups [128, 8, HB] in SBUF, spill to
    DRAM x^T with 1KB-per-partition-row descriptors. The staged tiles
    stay resident and serve as the identity mixer's (a=0) operands.
  - lhsT (block-diagonal weights expanded to 128x128 tiles) is built ON
    DEVICE from a 128KB compact table: lhsT[t, j, a, 2m+oo] =
    maskc[t, m] * Wc[t, j, a, oo], via per-partition-scalar multiplies
    split across the DVE / gpsimd / scalar engines. Saves 8.4MB of DMA.
  - Gather (per half, per output j-tile): one SWDGE dma_gather covering
    all 7 permuted mixers (num_idxs=896 <= descriptor-scratch limit),
    round-robined over SWDGE queues 1..3 (queue 0 carries the x^T
    spills).
  - Mix: per output j-tile, 8 accumulating 128x128 bf16 matmuls into a
    PSUM bank ([128, 512] f32); evacuated with the per-partition bias
    sum_a b[a] fused (alternating scalar/DVE), stored as y^T bf16 with
    one batched DMA per 4-j group on the scalar HWDGE queue.
"""

import os

import numpy as np
import ml_dtypes

import concourse.bass as bass
import concourse.bacc as bacc
import concourse.tile as tile
from concourse import library_config, mybir
from concourse.bass_utils import run_bass_kernel_spmd

B, D, A = 8192, 4096, 8
N_CORES = 8
BC = B // N_CORES          # 1024 batch rows per core
HB = BC // 2               # 512-wide batch half
NJ = D // 128              # 32 d-tiles of 128
JG = 4                     # j-tiles per mix group
NG = NJ // JG              # mix groups per half (8)
JSP = 8                    # j-tiles per phase-1 staging group
NSP = NJ // JSP            # staging groups per half (4)
NQ = 4                     # SWDGE queues
GCH = int(os.environ.get("GCH", "1"))  # j-tiles per gather call
JW = (A - 1) * 128 // 16   # idx columns per j-tile (56)

F32 = mybir.dt.float32
BF16 = mybir.dt.bfloat16
I16 = mybir.dt.int16

_GRAPH_CACHE = {}
_LAST_RESULTS = None


def _build_graph():
    nc = bacc.Bacc(None, num_swdge_queues=NQ)

    x_ext = nc.declare_dram_parameter("x", [BC, D], F32, isOutput=False)
    lhsT_ext = nc.declare_dram_parameter("lhsT", [NJ, 128, A * 128], BF16, isOutput=False)
    idx_ext = nc.declare_dram_parameter("idx", [128, NJ * JW], I16, isOutput=False)
    bsum_ext = nc.declare_dram_parameter("bsum", [128, NJ], F32, isOutput=False)
    ident_ext = nc.declare_dram_parameter("ident", [128, 128], F32, isOutput=False)
    yt_ext = nc.declare_dram_parameter("yt", [D, BC], BF16, isOutput=True)

    qn = [0]

    def next_q():
        q = qn[0]
        qn[0] = (q + 1) % NQ
        return q

    with tile.TileContext(nc) as tc:
        with (
            tc.tile_pool(name="const", bufs=1) as constp,
            tc.tile_pool(name="xin", bufs=4) as xinp,
            tc.tile_pool(name="st", bufs=6) as stp,
            tc.tile_pool(name="lhs", bufs=1) as lhsp,
            tc.tile_pool(name="g", bufs=2) as gthp,
            tc.tile_pool(name="y", bufs=2) as ybp,
            tc.tile_pool(name="pst", bufs=2, space="PSUM") as pstp,
            tc.tile_pool(name="psm", bufs=4, space="PSUM") as psmp,
            tc.tile_pool(name="dram", bufs=1, space="DRAM") as dramp,
        ):
            nc.gpsimd.load_library(library_config.mlp)

            ident = constp.tile([128, 128], F32)
            nc.sync.dma_start(out=ident[:], in_=ident_ext[:])
            idx_sb = constp.tile([128, NJ * JW], I16)
            nc.scalar.dma_start(out=idx_sb[:], in_=idx_ext[:])
            bsum_sb = constp.tile([128, NJ], F32)
            nc.scalar.dma_start(out=bsum_sb[:], in_=bsum_ext[:])

            # lhsT tiles resident in SBUF; the loads are issued on the sync
            # engine AFTER half 0's x loads (see below) so the head of
            # phase 1 gets the full DMA bandwidth
            lhs_all = lhsp.tile([128, NJ, A * 128], BF16)

            def load_lhs():
                for j0 in range(0, NJ, 4):
                    nc.sync.dma_start(
                        out=lhs_all[:, j0:j0 + 4, :],
                        in_=lhsT_ext[j0:j0 + 4].rearrange("j t m -> t j m"))

            yt_v = yt_ext[:].rearrange("(j p) b -> p j b", p=128)

            # per-half x^T DRAM buffers (rows of HB for the gathers)
            xt_d0 = dramp.tile([D, HB], BF16, tag="xt0")
            xt_d1 = dramp.tile([D, HB], BF16, tag="xt1")
            xt_d = [xt_d0, xt_d1]
            xt_v = [t[:].rearrange("(q p) b -> p q b", p=128) for t in xt_d]

            # staged x^T tiles, kept for the identity mixer: st[h][sg]
            st_tiles = [[None] * NSP for _ in range(2)]

            def phase1_group(h, sg):
                """load + PE transpose (f32) for j-tiles [8sg, 8sg+8) of
                batch half h; stage in SBUF (bf16) and spill to DRAM x^T."""
                st = stp.tile([128, JSP, HB], BF16, tag="st")
                st_tiles[h][sg] = st
                for bt in range(HB // 128):
                    b0 = h * HB + bt * 128
                    xin = xinp.tile([128, JSP * 128], F32, tag="xin")
                    nc.sync.dma_start(
                        out=xin[:],
                        in_=x_ext[b0:b0 + 128,
                                  sg * JSP * 128:(sg + 1) * JSP * 128])
                    pt = pstp.tile([128, JSP, 128], F32, tag="pst")
                    for jj in range(JSP):
                        nc.tensor.transpose(
                            pt[:, jj, :], xin[:, jj * 128:(jj + 1) * 128],
                            ident[:])
                    nc.vector.tensor_copy(
                        st[:, :, bt * 128:(bt + 1) * 128], pt[:])
                nc.gpsimd.dma_start(
                    out=xt_v[h][:, sg * JSP:(sg + 1) * JSP, :], in_=st[:])

            def mix_group(h, g):
                """gather (mixers 1..7) + mix + store for group g of half h."""
                gt = gthp.tile([128, JG * (A - 1), HB], BF16, tag="g")
                for c0 in range(0, JG, GCH):
                    j0 = g * JG + c0
                    nc.gpsimd.dma_gather(
                        out_ap=gt[:, c0 * (A - 1):(c0 + GCH) * (A - 1), :],
                        in_ap=xt_d[h][:],
                        idxs_ap=idx_sb[:, j0 * JW:(j0 + GCH) * JW],
                        num_idxs=GCH * (A - 1) * 128,
                        num_idxs_reg=GCH * (A - 1) * 128,
                        elem_size=HB,
                        queue_num=next_q(),
                    )
                st = st_tiles[h][g // (JSP // JG)]
                yb = ybp.tile([128, JG, HB], BF16, tag="y")
                for jc in range(JG):
                    j = g * JG + jc
                    pm = psmp.tile([128, HB], F32, tag="psm")
                    for a in range(A):
                        if a == 0:
                            rhs = st[:, (g % (JSP // JG)) * JG + jc, :]
                        else:
                            rhs = gt[:, jc * (A - 1) + (a - 1), :]
                        nc.tensor.matmul(
                            pm[:],
                            lhs_all[:, j, a * 128:(a + 1) * 128],
                            rhs,
                            start=(a == 0),
                            stop=(a == A - 1),
                        )
                    if jc % 2 == 0:
                        nc.scalar.activation(
                            yb[:, jc, :],
                            pm[:],
                            mybir.ActivationFunctionType.Identity,
                            bias=bsum_sb[:, j:j + 1],
                        )
                    else:
                        nc.vector.tensor_scalar_add(
                            yb[:, jc, :], pm[:], bsum_sb[:, j:j + 1])
                nc.scalar.dma_start(
                    out=yt_v[:, g * JG:(g + 1) * JG, h * HB:(h + 1) * HB],
                    in_=yb[:])

            for sg in range(NSP):
                phase1_group(0, sg)
            load_lhs()
            for sg in range(NSP):
                phase1_group(1, sg)
            for h in range(2):
                for g in range(NG):
                    mix_group(h, g)

    nc.compile()
    return nc


def _host_tables(W, b, perms):
    """Build the device-side constant tables from W/b/perms."""
    # lhsT[j, t, a, o]: weight applied to gathered row t (= x^T[perms[a, 128j+t]])
    # contributing to output row 128j+o.  Output 2n+oo uses inputs
    # perms[a, 2n+i] with weight W[a, n, i, oo]; within tile j, t = 2m+i,
    # o = 2m+oo for pair m = n - 64j.
    Wr = W.reshape(A, NJ, 64, 2, 2)
    lhsT = np.zeros((NJ, 128, A, 128), np.float32)
    m = np.arange(64)
    for i in range(2):
        for oo in range(2):
            # paired advanced indexing on axes 1 and 3 -> result axes [64, NJ, A]
            lhsT[:, 2 * m + i, :, 2 * m + oo] = Wr[:, :, :, i, oo].transpose(2, 1, 0)
    lhsT = np.ascontiguousarray(lhsT.reshape(NJ, 128, A * 128)).astype(ml_dtypes.bfloat16)

    # idx: per output j-tile, the concatenation over mixers a=1..7 of
    # perms[a, 128j : 128(j+1)], wrapped over 16 partitions (index i at
    # [i%16, i//16]) and replicated into each Q7 core's 16-partition group
    idx = np.zeros((128, NJ * JW), np.int16)
    for j in range(NJ):
        vec = np.concatenate([
            perms[a, j * 128:(j + 1) * 128] for a in range(1, A)
        ]).astype(np.int16)
        w16 = vec.reshape(JW, 16).T
        idx[:, j * JW:(j + 1) * JW] = np.tile(w16, (8, 1))

    bsum = np.ascontiguousarray(
        b.astype(np.float64).sum(axis=0).astype(np.float32).reshape(NJ, 128).T)
    ident = np.eye(128, dtype=np.float32)
    return lhsT, idx, bsum, ident


def _compact_tables(W):
    """Compact weight table + pair mask for on-device lhsT expansion."""
    Wr = W.reshape(A, NJ, 64, 2, 2)
    # Wc[t, j, a, oo] = W[a, 64j + t//2, t%2, oo]
    Wc = np.ascontiguousarray(Wr.transpose(2, 3, 1, 0, 4).reshape(128, NJ * A * 2))
    Wc = Wc.astype(ml_dtypes.bfloat16).astype(np.float32)
    t = np.arange(128)
    maskc = (np.arange(64)[None, :] == (t[:, None] // 2)).astype(ml_dtypes.bfloat16)
    return Wc, np.ascontiguousarray(maskc)


def kernel(x, W, b, perms):
    x = np.asarray(x, dtype=np.float32)
    W = np.asarray(W, dtype=np.float32)
    b = np.asarray(b, dtype=np.float32)
    perms = np.asarray(perms)

    lhsT, idx, bsum, ident = _host_tables(W, b, perms)

    if "nc" not in _GRAPH_CACHE:
        _GRAPH_CACHE["nc"] = _build_graph()
    nc = _GRAPH_CACHE["nc"]

    in_maps = []
    for c in range(N_CORES):
        in_maps.append({
            "x": np.ascontiguousarray(x[c * BC:(c + 1) * BC]),
            "lhsT": lhsT,
            "idx": idx,
            "bsum": bsum,
            "ident": ident,
        })

    res = run_bass_kernel_spmd(nc, in_maps, core_ids=list(range(N_CORES)))
    global _LAST_RESULTS
    _LAST_RESULTS = res
    y = np.concatenate(
        [np.asarray(res.results[c]["yt"], dtype=np.float32).T for c in range(N_CORES)],
        axis=0,
    )
    return np.ascontiguousarray(y)
